# revision 4
# baseline (speedup 1.0000x reference)
"""Trainium2 Bass kernel for MultiHeadedAttention with RoPE, v2.

Problem: b=4, n=2048, d=1024, H=16 heads, dk=64, rotary on first 32 dims
(interleaved pairs, theta=10000, lucidrains convention).

Sharding: 8 cores = 4 batches x 2 head-halves (data + head parallel).
Each core projects its 8 heads' Q/K/V for all 2048 rows, runs attention
for those heads, and produces a partial output projection (its heads'
contribution to all 1024 output features). Host adds the two partials
per batch (bo is passed as zeros to the hh=1 cores so it's counted once).

Device layout (features on partitions, all tensors transposed):
  - Per-head q/k feature order (host-permuted): [evens(16), pass(16),
    odds(16), pass(16)] so the RoPE pair rotation is a +-32-partition
    multiply-add. Projection bias is folded into an Activation-engine
    psum->sbuf bf16 copy, making all RoPE vector ops bf16/SBUF (4x DVE).
  - scores.T = K.T' @ Q.T' per head; the two heads of a 128-partition
    chunk use PE row tiles (0,0)/(64,0) which execute concurrently.
  - softmax: exp on Activation (the phase bottleneck; [128,1024] grain),
    normalizer Z from a ones-column appended to each head's V block.
  - attention output stays transposed; output projection streams it and
    a DVE bias-add writes bf16 partials to DRAM.
"""

import os

import numpy as np

B, N, D = 4, 2048, 1024
H, DK = 16, 64
HH = H // 2          # heads per core
DH = HH * DK         # 512 features per core
ROT, HALF = 32, 16
THETA = 10000.0
NCORES = 8
NQ = N  # full rows per core (test.py uses D x NQ for the fake result)

NKC = N // 128       # 16 key chunks
NDC = D // 128       # 8 contraction chunks
NFC = DH // 128      # 4 q/k feature chunks (= head pairs)
VW = HH * 65         # 520 V' columns (64 dims + ones per head)

_PROGRAM_CACHE = {}


def _build_program(mm_dtype_name="bfloat16"):
    import concourse.tile as tile
    from concourse import bacc, mybir
    from contextlib import ExitStack

    KLOOP = int(os.environ.get("KLOOP", "1"))  # hw-loop repeat (timing)

    f32 = mybir.dt.float32
    mmdt = getattr(mybir.dt, mm_dtype_name)
    AF = mybir.ActivationFunctionType
    ALU = mybir.AluOpType

    nc = bacc.Bacc("TRN2", target_bir_lowering=False)

    # DRAM I/O (per core). All *T tensors are feature-major (transposed).
    xqT = nc.dram_tensor("xqT", [D, N], mmdt, kind="ExternalInput")
    xkT = nc.dram_tensor("xkT", [D, N], mmdt, kind="ExternalInput")
    xvT = nc.dram_tensor("xvT", [D, N], mmdt, kind="ExternalInput")
    wqT = nc.dram_tensor("wqT", [D, DH], mmdt, kind="ExternalInput")
    wkT = nc.dram_tensor("wkT", [D, DH], mmdt, kind="ExternalInput")
    wvT = nc.dram_tensor("wvT", [D, VW], mmdt, kind="ExternalInput")
    wvb = nc.dram_tensor("wvb", [1, VW], mmdt, kind="ExternalInput")
    woT = nc.dram_tensor("woT", [DH, D], mmdt, kind="ExternalInput")
    bq_d = nc.dram_tensor("bq_d", [DH], f32, kind="ExternalInput")
    bk_d = nc.dram_tensor("bk_d", [DH], f32, kind="ExternalInput")
    bo_d = nc.dram_tensor("bo_d", [D], f32, kind="ExternalInput")
    cosT = nc.dram_tensor("cosT", [128, N], mmdt, kind="ExternalInput")
    # sin table with rows pre-permuted by the rotary partner map, so each
    # sin multiply reads psb and the table from the SAME partition base
    # (walrus requires equal bases when both inputs are in SBUF).
    sinPT = nc.dram_tensor("sinPT", [128, N], mmdt, kind="ExternalInput")
    outT = nc.dram_tensor("outT", [D, N], mmdt, kind="ExternalOutput")

    with ExitStack() as ctx:
        tc = ctx.enter_context(tile.TileContext(nc))

        const = ctx.enter_context(tc.tile_pool(name="const", bufs=1))
        dram = ctx.enter_context(tc.tile_pool(name="dram", bufs=4, space="DRAM"))

        # persistent sbuf tensors
        v_sb = const.tile([128, NKC, VW], mmdt)      # V' (keys, per-head 64+ones)
        q_sb = const.tile([128, NFC, N], mmdt)       # Q_rot.T
        k_sb = const.tile([128, NFC, N], mmdt)       # K_rot.T
        y_sb = const.tile([128, NFC, N], mmdt)       # Y.T (normalized attn out)
        bq_sb = const.tile([128, NFC], f32)
        bk_sb = const.tile([128, NFC], f32)
        bo_sb = const.tile([128, NDC], f32)
        cs_sb = const.tile([128, N], mmdt)
        sn_sb = const.tile([128, N], mmdt)
        ones1 = const.tile([1, 128], mmdt)
        wvb_sb = const.tile([1, VW], mmdt)
        nc.vector.memset(ones1[:], 1.0)
        nc.sync.dma_start(wvb_sb[:], wvb[:])

        _dmaq = [nc.sync, nc.gpsimd, nc.scalar]
        _dmaqi = [0]

        def dma_rr(dst, src_ap):
            eng = _dmaq[_dmaqi[0] % len(_dmaq)]
            _dmaqi[0] += 1
            eng.dma_start(dst, src_ap)

        def load_chunked(dst_tile, src_t, nchunks, splits=8):
            # dst [128, nchunks, cols]; src (c p) cols layout
            per = nchunks // splits if nchunks % splits == 0 else 1
            if per == 0:
                per = 1
            c = 0
            while c < nchunks:
                n = min(per, nchunks - c)
                dma_rr(
                    dst_tile[:, c:c + n, :],
                    src_t[c * 128:(c + n) * 128, :].rearrange(
                        "(c p) r -> p c r", p=128),
                )
                c += n

        nc.sync.dma_start(bq_sb[:], bq_d.rearrange("(c p) -> p c", p=128))
        nc.sync.dma_start(bk_sb[:], bk_d.rearrange("(c p) -> p c", p=128))
        nc.sync.dma_start(bo_sb[:], bo_d.rearrange("(c p) -> p c", p=128))
        nc.sync.dma_start(cs_sb[:], cosT[:])
        nc.sync.dma_start(sn_sb[:], sinPT[:])

        def phase_v():
            with tc.tile_pool(name="vphase", bufs=1) as vp, \
                 tc.tile_pool(name="vmerge", bufs=3) as vps_merge, \
                 tc.tile_pool(name="vpsum", bufs=4, space="PSUM") as vps:
                xv_sb = vp.tile([128, NDC, N], mmdt)
                wv_sb = vp.tile([128, NDC, VW], mmdt)
                load_chunked(xv_sb, xvT, NDC)
                load_chunked(wv_sb, wvT, NDC)
                for kc in range(NKC):
                    for nf in range(2):  # 520 = 2 * 260
                        cols = slice(nf * 260, (nf + 1) * 260)
                        # contraction split into 64-row halves, alternating
                        # row groups (LDWEIGHTS overlap + tile concurrency)
                        ps_lo = vps.tile([128, 260], f32, tag="vlo")
                        ps_hi = vps.tile([128, 260], f32, tag="vhi")
                        for dc in range(NDC):
                            for ps, r0 in ((ps_lo, 0), (ps_hi, 64)):
                                nc.tensor.matmul(
                                    ps[:],
                                    lhsT=xv_sb[r0:r0 + 64, dc,
                                               kc * 128:(kc + 1) * 128],
                                    rhs=wv_sb[r0:r0 + 64, dc, cols],
                                    start=(dc == 0),
                                    stop=(ps is ps_hi and dc == NDC - 1),
                                )
                        # bias + ones row (K=1): V' gets +bv and the Z column
                        nc.tensor.matmul(
                            ps_lo[:],
                            lhsT=ones1[:, 0:128],
                            rhs=wvb_sb[:, cols],
                            start=False,
                            stop=True,
                        )
                        vh = vps_merge.tile([128, 260], f32, tag="vh")
                        nc.scalar.activation(vh[:], ps_hi[:], AF.Identity)
                        nc.vector.tensor_add(
                            v_sb[:, kc, cols], ps_lo[:], vh[:]
                        )

        # Persistent zeroed sin-term temporaries: pass rows stay zero forever;
        # only the 4x16 rotary rows are rewritten each block.
        tmpS_tiles = [
            const.tile([128, 1024], mmdt, tag=f"tmpS{i}", name=f"tmpS{i}")
            for i in (0, 1)
        ]
        for t in tmpS_tiles:
            nc.vector.memset(t[:], 0.0)

        # Per-head feature layout (after the host permutation):
        #   [0:16) evens, [16:32) pass, [32:48) odds, [48:64) pass
        # so rotary partners are at +-32 partitions within each 64-row head.
        def proj_rope(x_sb, w_sb, b_sb, dst_sb, rope_pool, rope_psum):
            for fc in range(NFC):
                for rb in range(N // 1024):
                    r0 = rb * 1024
                    ps = rope_psum.tile([128, 1024], f32, tag="qk_ps")
                    for dc in range(NDC):
                        for h512 in range(2):
                            nc.tensor.matmul(
                                ps[:, h512 * 512:(h512 + 1) * 512],
                                lhsT=w_sb[:, dc, fc * 128:(fc + 1) * 128],
                                rhs=x_sb[:, dc, r0 + h512 * 512:r0 + (h512 + 1) * 512],
                                start=(dc == 0),
                                stop=(dc == NDC - 1),
                            )
                    # psum -> sbuf bf16 with bias folded (Activation engine)
                    psb = rope_pool.tile([128, 1024], mmdt, tag="psb")
                    nc.scalar.activation(
                        psb[:], ps[:], AF.Identity, bias=b_sb[:, fc:fc + 1]
                    )
                    tmpC = rope_pool.tile([128, 1024], mmdt, tag="tmpC")
                    tmpS = tmpS_tiles[(fc + rb) % 2]
                    nc.vector.tensor_mul(
                        tmpC[:], psb[:], cs_sb[:, r0:r0 + 1024]
                    )
                    # sin part: partner rows at +-32; sn_sb is partner-
                    # permuted so in0/in1 share a partition base.
                    for h2 in (0, 64):
                        nc.vector.tensor_mul(
                            tmpS[h2:h2 + 16, :],
                            psb[h2 + 32:h2 + 48, :],
                            sn_sb[h2 + 32:h2 + 48, r0:r0 + 1024],
                        )
                        nc.vector.tensor_mul(
                            tmpS[h2 + 32:h2 + 48, :],
                            psb[h2:h2 + 16, :],
                            sn_sb[h2:h2 + 16, r0:r0 + 1024],
                        )
                    nc.vector.tensor_add(
                        dst_sb[:, fc, r0:r0 + 1024], tmpC[:], tmpS[:]
                    )

        def phase_q():
            with tc.tile_pool(name="qphase", bufs=1) as qp, \
                 tc.tile_pool(name="qrope", bufs=3) as qrp, \
                 tc.tile_pool(name="qpsum", bufs=2, space="PSUM") as qps:
                xq_sb = qp.tile([128, NDC, N], mmdt)
                wq_sb = qp.tile([128, NDC, DH], mmdt)
                load_chunked(xq_sb, xqT, NDC)
                load_chunked(wq_sb, wqT, NDC)
                proj_rope(xq_sb, wq_sb, bq_sb, q_sb, qrp, qps)

        def phase_k():
            with tc.tile_pool(name="kphase", bufs=1) as kp, \
                 tc.tile_pool(name="krope", bufs=3) as krp, \
                 tc.tile_pool(name="kpsum", bufs=2, space="PSUM") as kps:
                xk_sb = kp.tile([128, NDC, N], mmdt)
                wk_sb = kp.tile([128, NDC, DH], mmdt)
                load_chunked(xk_sb, xkT, NDC)
                load_chunked(wk_sb, wkT, NDC)
                proj_rope(xk_sb, wk_sb, bk_sb, k_sb, krp, kps)

        QB = 512  # query block (1 PSUM bank) -- everything double-buffered
        NQB = N // QB

        def attn_scores(p, q0, kc, spool):
            """Head-pair score matmuls; the two PE row tiles run concurrently."""
            ps_A = spool.tile([128, QB], f32, tag="sA")
            ps_B = spool.tile([128, QB], f32, tag="sB")
            for ps, r0 in ((ps_A, 0), (ps_B, 64)):
                nc.tensor.matmul(
                    ps[:],
                    lhsT=k_sb[r0:r0 + 64, p, kc * 128:(kc + 1) * 128],
                    rhs=q_sb[r0:r0 + 64, p, q0:q0 + QB],
                    start=True, stop=True,
                )
            return ps_A, ps_B

        def phase_attn():
            # PSUM: sA/sB double-buffered (4 banks) + poA/poB double-buffered
            # (4 banks) = 8. Full double-buffering keeps the PE queue fed so
            # the tensor engine streams continuously (HAM stays at full clock).
            with tc.tile_pool(name="spsum", bufs=2, space="PSUM") as sps, \
                 tc.tile_pool(name="opsum", bufs=1, space="PSUM") as ops_pool, \
                 tc.tile_pool(name="ppool", bufs=4) as pp, \
                 tc.tile_pool(name="npool", bufs=3) as npl:
                for p in range(NFC):
                    hA, hB = 2 * p, 2 * p + 1
                    for qn in range(NQB):
                        q0 = qn * QB
                        # AV contraction split into 64-row halves with
                        # separate accumulators: every consecutive PE matmul
                        # alternates row group, so LDWEIGHTS pulls ahead and
                        # row tiles execute concurrently.
                        po = {
                            (h, r0): ops_pool.tile(
                                [65, QB], f32, tag=f"po{i}{j}",
                                name=f"po{i}{j}")
                            for i, h in enumerate((hA, hB))
                            for j, r0 in enumerate((0, 64))
                        }
                        def issue_av(pt_A, pt_B, kc):
                            for pt, h in ((pt_A, hA), (pt_B, hB)):
                                for r0 in (0, 64):
                                    nc.tensor.matmul(
                                        po[(h, r0)][:],
                                        lhsT=v_sb[r0:r0 + 64, kc,
                                                  h * 65:(h + 1) * 65],
                                        rhs=pt[r0:r0 + 64, :],
                                        start=(kc == 0),
                                        stop=(kc == NKC - 1),
                                    )

                        # software pipeline: AV for kc-1 issues after the
                        # exp+scores of kc, so AV never waits on a sem from
                        # an exp that just finished.
                        ps_A, ps_B = attn_scores(p, q0, 0, sps)
                        pending = []
                        for kc in range(NKC):
                            pt_A = pp.tile([128, QB], mmdt, tag="ptA")
                            pt_B = pp.tile([128, QB], mmdt, tag="ptB")
                            nc.scalar.activation(
                                pt_A[:], ps_A[:], AF.Exp, scale=1.0 / 8.0)
                            nc.scalar.activation(
                                pt_B[:], ps_B[:], AF.Exp, scale=1.0 / 8.0)
                            if kc + 1 < NKC:
                                ps_A, ps_B = attn_scores(p, q0, kc + 1, sps)
                            pending.append((pt_A, pt_B, kc))
                            if len(pending) > 1:
                                issue_av(*pending.pop(0))
                        for args in pending:
                            issue_av(*args)
                        # merge halves + normalize: y.T = sum/Z. The copy/add
                        # also frees the PSUM banks without waiting on the
                        # broadcast DMA roundtrip.
                        for h, hb in ((hA, 0), (hB, 64)):
                            yh = npl.tile([65, QB], f32, tag="yh")
                            nc.scalar.activation(
                                yh[:], po[(h, 64)][:], AF.Identity)
                            yt = npl.tile([65, QB], f32, tag="yt")
                            nc.vector.tensor_add(yt[:], po[(h, 0)][:], yh[:])
                            rz = npl.tile([1, QB], f32, tag="rz")
                            nc.vector.reciprocal(rz[:], yt[64:65, :])
                            rz_dram = dram.tile([1, QB], f32, tag="rzd")
                            nc.gpsimd.dma_start(rz_dram[:], rz[:])
                            rzb = npl.tile([64, QB], f32, tag="rzb")
                            nc.gpsimd.dma_start(
                                rzb[:], rz_dram[:].to_broadcast([64, QB]))
                            nc.vector.tensor_mul(
                                y_sb[hb:hb + 64, p, q0:q0 + QB],
                                yt[0:64, :], rzb[:],
                            )

        # prefetch output-projection weights at program start (overlaps
        # earlier compute; avoids a load stall after attention)
        wo_sb = const.tile([128, NFC, D], mmdt)
        load_chunked(wo_sb, woT, NFC)

        def phase_out():
            with tc.tile_pool(name="owork", bufs=3) as owork, \
                 tc.tile_pool(name="opsum2", bufs=4, space="PSUM") as ops2:
                for dmc in range(NDC):
                    ob = owork.tile([128, N], mmdt, tag="ob")
                    for rn in range(N // 512):
                        ps_lo = ops2.tile([128, 512], f32, tag="olo")
                        ps_hi = ops2.tile([128, 512], f32, tag="ohi")
                        for fc in range(NFC):
                            for ps, r0 in ((ps_lo, 0), (ps_hi, 64)):
                                nc.tensor.matmul(
                                    ps[:],
                                    lhsT=wo_sb[r0:r0 + 64, fc,
                                               dmc * 128:(dmc + 1) * 128],
                                    rhs=y_sb[r0:r0 + 64, fc,
                                             rn * 512:(rn + 1) * 512],
                                    start=(fc == 0),
                                    stop=(fc == NFC - 1),
                                )
                        oh = owork.tile([128, 512], f32, tag="oh")
                        nc.scalar.activation(oh[:], ps_hi[:], AF.Identity)
                        nc.vector.scalar_tensor_tensor(
                            ob[:, rn * 512:(rn + 1) * 512], ps_lo[:],
                            bo_sb[:, dmc:dmc + 1], oh[:],
                            op0=ALU.add, op1=ALU.add)
                        # stream each 512-col stripe out as soon as it's done
                        dma_rr(
                            outT[dmc * 128:(dmc + 1) * 128,
                                 rn * 512:(rn + 1) * 512],
                            ob[:, rn * 512:(rn + 1) * 512])

        PHASES = int(os.environ.get("KPHASES", "9"))  # debug bisect knob

        def all_phases():
            if PHASES >= 1:
                phase_v()
            if PHASES >= 2:
                phase_q()
            if PHASES >= 3:
                phase_k()
            if PHASES >= 4:
                phase_attn()
            else:
                nc.vector.memset(y_sb[:], 0.0)
            if PHASES >= 5:
                phase_out()
            else:
                with tc.tile_pool(name="dummy", bufs=1) as dp:
                    zb = dp.tile([128, N], mmdt)
                    nc.vector.memset(zb[:], 0.0)
                    for dmc in range(NDC):
                        nc.sync.dma_start(
                            outT[dmc * 128:(dmc + 1) * 128, :], zb[:])

        if KLOOP > 1:
            with tc.For_i(0, KLOOP, 1):
                all_phases()
        else:
            all_phases()

    nc.compile()
    return nc


def _rope_tables(positions):
    """cos/sin tables [128, len(positions)] for the permuted transposed
    layout: partition p (within a 2-head feature chunk), j = p % 64:
    j<16: freq j (cos, -sin); 32<=j<48: freq j-32 (cos, +sin); else (1, 0)."""
    inv_freq = 1.0 / (THETA ** (np.arange(0, ROT, 2, dtype=np.float64) / ROT))  # [16]
    t = np.asarray(positions, dtype=np.float64)
    ang = t[None, :] * inv_freq[:, None]  # [16, nt]
    c, s = np.cos(ang), np.sin(ang)
    cos_tab = np.ones((128, len(positions)), dtype=np.float64)
    sin_tab = np.zeros((128, len(positions)), dtype=np.float64)
    for h2 in (0, 64):
        cos_tab[h2:h2 + 16] = c
        cos_tab[h2 + 32:h2 + 48] = c
        sin_tab[h2:h2 + 16] = -s
        sin_tab[h2 + 32:h2 + 48] = s
    return cos_tab.astype(np.float32), sin_tab.astype(np.float32)


def _head_perm(nfeat):
    """Feature permutation applied per head: within each head's 64 outputs
    -> [evens(16), pass 32:48, odds(16), pass 48:64]."""
    out = np.empty(nfeat, dtype=np.int64)
    for h in range(nfeat // DK):
        base = h * DK
        out[base:base + HALF] = base + np.arange(0, ROT, 2)
        out[base + HALF:base + ROT] = base + np.arange(ROT, ROT + HALF)
        out[base + ROT:base + ROT + HALF] = base + np.arange(1, ROT, 2)
        out[base + ROT + HALF:base + DK] = base + np.arange(ROT + HALF, DK)
    return out


def _prep_inputs(query, key, value, Wq, bq, Wk, bk, Wv, bv, Wo, bo,
                 mm_dtype_name="bfloat16"):
    import ml_dtypes

    np_mm = ml_dtypes.bfloat16 if mm_dtype_name == "bfloat16" else np.float32

    query = np.asarray(query, np.float32)
    key = np.asarray(key, np.float32)
    value = np.asarray(value, np.float32)
    Wq, bq = np.asarray(Wq, np.float32), np.asarray(bq, np.float32)
    Wk, bk = np.asarray(Wk, np.float32), np.asarray(bk, np.float32)
    Wv, bv = np.asarray(Wv, np.float32), np.asarray(bv, np.float32)
    Wo, bo = np.asarray(Wo, np.float32), np.asarray(bo, np.float32)

    perm = _head_perm(DH)
    cos_all, sin_all = _rope_tables(np.arange(N))
    # partner-permuted sin table (see sinPT comment in _build_program)
    pmap = np.arange(128)
    for h2 in (0, 64):
        pmap[h2:h2 + 16] = np.arange(h2 + 32, h2 + 48)
        pmap[h2 + 32:h2 + 48] = np.arange(h2, h2 + 16)
    sinP_all = sin_all[pmap]

    halves = []
    for hh in range(2):
        rows = slice(hh * DH, (hh + 1) * DH)  # head-feature rows of this half
        Wq_h, bq_h = Wq[rows][perm], bq[rows][perm]
        Wk_h, bk_h = Wk[rows][perm], bk[rows][perm]
        wqT = np.ascontiguousarray(Wq_h.T).astype(np_mm)
        wkT = np.ascontiguousarray(Wk_h.T).astype(np_mm)

        # W_v' : [D, HH*65] plus a separate bias/ones row wvb [1, HH*65]
        wvT = np.zeros((D, VW), np.float32)
        wvb = np.zeros((1, VW), np.float32)
        for h in range(HH):
            cols = slice(h * 65, h * 65 + 64)
            grows = slice(hh * DH + h * DK, hh * DH + (h + 1) * DK)
            wvT[:, cols] = Wv[grows, :].T
            wvb[0, cols] = bv[grows]
            wvb[0, h * 65 + 64] = 1.0

        # woT rows follow the y_sb layout: chunk fc holds heads (2fc, 2fc+1)
        wo_rows = np.empty((DH, D), np.float32)
        for fc in range(NFC):
            for sub in range(2):
                h = 2 * fc + sub
                grows = slice(hh * DH + h * DK, hh * DH + (h + 1) * DK)
                wo_rows[fc * 128 + sub * 64:fc * 128 + (sub + 1) * 64] = \
                    Wo[:, grows].T
        halves.append({
            "wqT": wqT, "wkT": wkT,
            "wvT": wvT.astype(np_mm), "wvb": wvb.astype(np_mm),
            "woT": np.ascontiguousarray(wo_rows).astype(np_mm),
            "bq_d": bq_h, "bk_d": bk_h,
            "bo_d": bo if hh == 0 else np.zeros_like(bo),
        })

    in_maps = []
    for core in range(NCORES):
        b, hh = core // 2, core % 2
        m = {
            "xqT": np.ascontiguousarray(query[b].T).astype(np_mm),
            "xkT": np.ascontiguousarray(key[b].T).astype(np_mm),
            "xvT": np.ascontiguousarray(value[b].T).astype(np_mm),
            "cosT": cos_all.astype(np_mm),
            "sinPT": sinP_all.astype(np_mm),
        }
        m.update(halves[hh])
        in_maps.append(m)
    return in_maps


def kernel(query, key, value, Wq, bq, Wk, bk, Wv, bv, Wo, bo):
    from concourse import bass_utils

    mm_dtype_name = "bfloat16"
    if mm_dtype_name not in _PROGRAM_CACHE:
        _PROGRAM_CACHE[mm_dtype_name] = _build_program(mm_dtype_name)
    nc = _PROGRAM_CACHE[mm_dtype_name]

    in_maps = _prep_inputs(query, key, value, Wq, bq, Wk, bk, Wv, bv, Wo, bo,
                           mm_dtype_name)

    res = bass_utils.run_bass_kernel_spmd(
        nc, in_maps, core_ids=list(range(NCORES))
    )

    out = np.empty((B, N, D), np.float32)
    for b in range(B):
        p0 = np.asarray(res.results[2 * b]["outT"], np.float32)
        p1 = np.asarray(res.results[2 * b + 1]["outT"], np.float32)
        out[b] = (p0 + p1).T
    return out


# revision 5
# speedup vs baseline: 1.0002x; 1.0002x over previous
"""Trainium2 Bass kernel for MultiHeadedAttention with RoPE, v2.

Problem: b=4, n=2048, d=1024, H=16 heads, dk=64, rotary on first 32 dims
(interleaved pairs, theta=10000, lucidrains convention).

Sharding: 8 cores = 4 batches x 2 head-halves (data + head parallel).
Each core projects its 8 heads' Q/K/V for all 2048 rows, runs attention
for those heads, and produces a partial output projection (its heads'
contribution to all 1024 output features). Host adds the two partials
per batch (bo is passed as zeros to the hh=1 cores so it's counted once).

Device layout (features on partitions, all tensors transposed):
  - Per-head q/k feature order (host-permuted): [evens(16), pass(16),
    odds(16), pass(16)] so the RoPE pair rotation is a +-32-partition
    multiply-add. Projection bias is folded into an Activation-engine
    psum->sbuf bf16 copy, making all RoPE vector ops bf16/SBUF (4x DVE).
  - scores.T = K.T' @ Q.T' per head; the two heads of a 128-partition
    chunk use PE row tiles (0,0)/(64,0) which execute concurrently.
  - softmax: exp on Activation (the phase bottleneck; [128,1024] grain),
    normalizer Z from a ones-column appended to each head's V block.
  - attention output stays transposed; output projection streams it and
    a DVE bias-add writes bf16 partials to DRAM.
"""

import os

import numpy as np

B, N, D = 4, 2048, 1024
H, DK = 16, 64
HH = H // 2          # heads per core
DH = HH * DK         # 512 features per core
ROT, HALF = 32, 16
THETA = 10000.0
NCORES = 8
NQ = N  # full rows per core (test.py uses D x NQ for the fake result)

NKC = N // 128       # 16 key chunks
NDC = D // 128       # 8 contraction chunks
NFC = DH // 128      # 4 q/k feature chunks (= head pairs)
VW = HH * 65         # 520 V' columns (64 dims + ones per head)

_PROGRAM_CACHE = {}


def _build_program(mm_dtype_name="bfloat16"):
    import concourse.tile as tile
    from concourse import bacc, mybir
    from contextlib import ExitStack

    KLOOP = int(os.environ.get("KLOOP", "1"))  # hw-loop repeat (timing)

    f32 = mybir.dt.float32
    mmdt = getattr(mybir.dt, mm_dtype_name)
    AF = mybir.ActivationFunctionType
    ALU = mybir.AluOpType

    nc = bacc.Bacc("TRN2", target_bir_lowering=False)

    # DRAM I/O (per core). All *T tensors are feature-major (transposed).
    xqT = nc.dram_tensor("xqT", [D, N], mmdt, kind="ExternalInput")
    xkT = nc.dram_tensor("xkT", [D, N], mmdt, kind="ExternalInput")
    xvT = nc.dram_tensor("xvT", [D, N], mmdt, kind="ExternalInput")
    wqT = nc.dram_tensor("wqT", [D, DH], mmdt, kind="ExternalInput")
    wkT = nc.dram_tensor("wkT", [D, DH], mmdt, kind="ExternalInput")
    wvT = nc.dram_tensor("wvT", [D, VW], mmdt, kind="ExternalInput")
    wvb = nc.dram_tensor("wvb", [1, VW], mmdt, kind="ExternalInput")
    woT = nc.dram_tensor("woT", [DH, D], mmdt, kind="ExternalInput")
    bq_d = nc.dram_tensor("bq_d", [DH], f32, kind="ExternalInput")
    bk_d = nc.dram_tensor("bk_d", [DH], f32, kind="ExternalInput")
    bo_d = nc.dram_tensor("bo_d", [D], f32, kind="ExternalInput")
    cosT = nc.dram_tensor("cosT", [128, N], mmdt, kind="ExternalInput")
    # sin table with rows pre-permuted by the rotary partner map, so each
    # sin multiply reads psb and the table from the SAME partition base
    # (walrus requires equal bases when both inputs are in SBUF).
    sinPT = nc.dram_tensor("sinPT", [128, N], mmdt, kind="ExternalInput")
    outT = nc.dram_tensor("outT", [D, N], mmdt, kind="ExternalOutput")

    with ExitStack() as ctx:
        tc = ctx.enter_context(tile.TileContext(nc))

        const = ctx.enter_context(tc.tile_pool(name="const", bufs=1))
        dram = ctx.enter_context(tc.tile_pool(name="dram", bufs=4, space="DRAM"))

        # persistent sbuf tensors
        v_sb = const.tile([128, NKC, VW], mmdt)      # V' (keys, per-head 64+ones)
        q_sb = const.tile([128, NFC, N], mmdt)       # Q_rot.T
        k_sb = const.tile([128, NFC, N], mmdt)       # K_rot.T
        y_sb = const.tile([128, NFC, N], mmdt)       # Y.T (normalized attn out)
        bq_sb = const.tile([128, NFC], f32)
        bk_sb = const.tile([128, NFC], f32)
        bo_sb = const.tile([128, NDC], f32)
        cs_sb = const.tile([128, N], mmdt)
        sn_sb = const.tile([128, N], mmdt)
        ones1 = const.tile([1, 128], mmdt)
        wvb_sb = const.tile([1, VW], mmdt)
        nc.vector.memset(ones1[:], 1.0)
        nc.sync.dma_start(wvb_sb[:], wvb[:])

        _dmaq = [nc.sync, nc.gpsimd, nc.scalar]
        _dmaqi = [0]

        def dma_rr(dst, src_ap):
            eng = _dmaq[_dmaqi[0] % len(_dmaq)]
            _dmaqi[0] += 1
            eng.dma_start(dst, src_ap)

        def load_chunked(dst_tile, src_t, nchunks, splits=8):
            # dst [128, nchunks, cols]; src (c p) cols layout
            per = nchunks // splits if nchunks % splits == 0 else 1
            if per == 0:
                per = 1
            c = 0
            while c < nchunks:
                n = min(per, nchunks - c)
                dma_rr(
                    dst_tile[:, c:c + n, :],
                    src_t[c * 128:(c + n) * 128, :].rearrange(
                        "(c p) r -> p c r", p=128),
                )
                c += n

        nc.sync.dma_start(bq_sb[:], bq_d.rearrange("(c p) -> p c", p=128))
        nc.sync.dma_start(bk_sb[:], bk_d.rearrange("(c p) -> p c", p=128))
        nc.sync.dma_start(bo_sb[:], bo_d.rearrange("(c p) -> p c", p=128))
        nc.sync.dma_start(cs_sb[:], cosT[:])
        nc.sync.dma_start(sn_sb[:], sinPT[:])

        def phase_v():
            with tc.tile_pool(name="vphase", bufs=1) as vp, \
                 tc.tile_pool(name="vmerge", bufs=4) as vps_merge, \
                 tc.tile_pool(name="vpsum", bufs=4, space="PSUM") as vps:
                xv_sb = vp.tile([128, NDC, N], mmdt)
                wv_sb = vp.tile([128, NDC, VW], mmdt)
                load_chunked(xv_sb, xvT, NDC)
                load_chunked(wv_sb, wvT, NDC)
                for kc in range(NKC):
                    for nf in range(2):  # 520 = 2 * 260
                        cols = slice(nf * 260, (nf + 1) * 260)
                        # contraction split into 64-row halves, alternating
                        # row groups (LDWEIGHTS overlap + tile concurrency)
                        ps_lo = vps.tile([128, 260], f32, tag="vlo")
                        ps_hi = vps.tile([128, 260], f32, tag="vhi")
                        for dc in range(NDC):
                            for ps, r0 in ((ps_lo, 0), (ps_hi, 64)):
                                nc.tensor.matmul(
                                    ps[:],
                                    lhsT=xv_sb[r0:r0 + 64, dc,
                                               kc * 128:(kc + 1) * 128],
                                    rhs=wv_sb[r0:r0 + 64, dc, cols],
                                    start=(dc == 0),
                                    stop=(ps is ps_hi and dc == NDC - 1),
                                )
                        # bias + ones row (K=1): V' gets +bv and the Z column
                        nc.tensor.matmul(
                            ps_lo[:],
                            lhsT=ones1[:, 0:128],
                            rhs=wvb_sb[:, cols],
                            start=False,
                            stop=True,
                        )
                        vh = vps_merge.tile([128, 260], f32, tag="vh")
                        nc.scalar.activation(vh[:], ps_hi[:], AF.Identity)
                        nc.vector.tensor_add(
                            v_sb[:, kc, cols], ps_lo[:], vh[:]
                        )

        # Persistent zeroed sin-term temporaries: pass rows stay zero forever;
        # only the 4x16 rotary rows are rewritten each block.
        tmpS_tiles = [
            const.tile([128, 1024], mmdt, tag=f"tmpS{i}", name=f"tmpS{i}")
            for i in (0, 1)
        ]
        for t in tmpS_tiles:
            nc.vector.memset(t[:], 0.0)

        # Per-head feature layout (after the host permutation):
        #   [0:16) evens, [16:32) pass, [32:48) odds, [48:64) pass
        # so rotary partners are at +-32 partitions within each 64-row head.
        def proj_rope(x_sb, w_sb, b_sb, dst_sb, rope_pool, rope_psum):
            for fc in range(NFC):
                for rb in range(N // 1024):
                    r0 = rb * 1024
                    ps = rope_psum.tile([128, 1024], f32, tag="qk_ps")
                    for dc in range(NDC):
                        for h512 in range(2):
                            nc.tensor.matmul(
                                ps[:, h512 * 512:(h512 + 1) * 512],
                                lhsT=w_sb[:, dc, fc * 128:(fc + 1) * 128],
                                rhs=x_sb[:, dc, r0 + h512 * 512:r0 + (h512 + 1) * 512],
                                start=(dc == 0),
                                stop=(dc == NDC - 1),
                            )
                    # psum -> sbuf bf16 with bias folded (Activation engine)
                    psb = rope_pool.tile([128, 1024], mmdt, tag="psb")
                    nc.scalar.activation(
                        psb[:], ps[:], AF.Identity, bias=b_sb[:, fc:fc + 1]
                    )
                    tmpC = rope_pool.tile([128, 1024], mmdt, tag="tmpC")
                    tmpS = tmpS_tiles[(fc + rb) % 2]
                    nc.vector.tensor_mul(
                        tmpC[:], psb[:], cs_sb[:, r0:r0 + 1024]
                    )
                    # sin part: partner rows at +-32; sn_sb is partner-
                    # permuted so in0/in1 share a partition base.
                    for h2 in (0, 64):
                        nc.vector.tensor_mul(
                            tmpS[h2:h2 + 16, :],
                            psb[h2 + 32:h2 + 48, :],
                            sn_sb[h2 + 32:h2 + 48, r0:r0 + 1024],
                        )
                        nc.vector.tensor_mul(
                            tmpS[h2 + 32:h2 + 48, :],
                            psb[h2:h2 + 16, :],
                            sn_sb[h2:h2 + 16, r0:r0 + 1024],
                        )
                    nc.vector.tensor_add(
                        dst_sb[:, fc, r0:r0 + 1024], tmpC[:], tmpS[:]
                    )

        def phase_q():
            with tc.tile_pool(name="qphase", bufs=1) as qp, \
                 tc.tile_pool(name="qrope", bufs=4) as qrp, \
                 tc.tile_pool(name="qpsum", bufs=2, space="PSUM") as qps:
                xq_sb = qp.tile([128, NDC, N], mmdt)
                wq_sb = qp.tile([128, NDC, DH], mmdt)
                load_chunked(xq_sb, xqT, NDC)
                load_chunked(wq_sb, wqT, NDC)
                proj_rope(xq_sb, wq_sb, bq_sb, q_sb, qrp, qps)

        def phase_k():
            with tc.tile_pool(name="kphase", bufs=1) as kp, \
                 tc.tile_pool(name="krope", bufs=4) as krp, \
                 tc.tile_pool(name="kpsum", bufs=2, space="PSUM") as kps:
                xk_sb = kp.tile([128, NDC, N], mmdt)
                wk_sb = kp.tile([128, NDC, DH], mmdt)
                load_chunked(xk_sb, xkT, NDC)
                load_chunked(wk_sb, wkT, NDC)
                proj_rope(xk_sb, wk_sb, bk_sb, k_sb, krp, kps)

        QB = 512  # query block (1 PSUM bank) -- everything double-buffered
        NQB = N // QB

        def attn_scores(p, q0, kc, spool):
            """Head-pair score matmuls; the two PE row tiles run concurrently."""
            ps_A = spool.tile([128, QB], f32, tag="sA")
            ps_B = spool.tile([128, QB], f32, tag="sB")
            for ps, r0 in ((ps_A, 0), (ps_B, 64)):
                nc.tensor.matmul(
                    ps[:],
                    lhsT=k_sb[r0:r0 + 64, p, kc * 128:(kc + 1) * 128],
                    rhs=q_sb[r0:r0 + 64, p, q0:q0 + QB],
                    start=True, stop=True,
                )
            return ps_A, ps_B

        def phase_attn():
            # PSUM: sA/sB double-buffered (4 banks) + poA/poB double-buffered
            # (4 banks) = 8. Full double-buffering keeps the PE queue fed so
            # the tensor engine streams continuously (HAM stays at full clock).
            with tc.tile_pool(name="spsum", bufs=2, space="PSUM") as sps, \
                 tc.tile_pool(name="opsum", bufs=1, space="PSUM") as ops_pool, \
                 tc.tile_pool(name="ppool", bufs=6) as pp, \
                 tc.tile_pool(name="npool", bufs=4) as npl:
                for p in range(NFC):
                    hA, hB = 2 * p, 2 * p + 1
                    for qn in range(NQB):
                        q0 = qn * QB
                        # AV contraction split into 64-row halves with
                        # separate accumulators: every consecutive PE matmul
                        # alternates row group, so LDWEIGHTS pulls ahead and
                        # row tiles execute concurrently.
                        po = {
                            (h, r0): ops_pool.tile(
                                [65, QB], f32, tag=f"po{i}{j}",
                                name=f"po{i}{j}")
                            for i, h in enumerate((hA, hB))
                            for j, r0 in enumerate((0, 64))
                        }
                        def issue_av(pt_A, pt_B, kc):
                            for pt, h in ((pt_A, hA), (pt_B, hB)):
                                for r0 in (0, 64):
                                    nc.tensor.matmul(
                                        po[(h, r0)][:],
                                        lhsT=v_sb[r0:r0 + 64, kc,
                                                  h * 65:(h + 1) * 65],
                                        rhs=pt[r0:r0 + 64, :],
                                        start=(kc == 0),
                                        stop=(kc == NKC - 1),
                                    )

                        # software pipeline: AV for kc-1 issues after the
                        # exp+scores of kc, so AV never waits on a sem from
                        # an exp that just finished.
                        ps_A, ps_B = attn_scores(p, q0, 0, sps)
                        pending = []
                        for kc in range(NKC):
                            pt_A = pp.tile([128, QB], mmdt, tag="ptA")
                            pt_B = pp.tile([128, QB], mmdt, tag="ptB")
                            nc.scalar.activation(
                                pt_A[:], ps_A[:], AF.Exp, scale=1.0 / 8.0)
                            nc.scalar.activation(
                                pt_B[:], ps_B[:], AF.Exp, scale=1.0 / 8.0)
                            if kc + 1 < NKC:
                                ps_A, ps_B = attn_scores(p, q0, kc + 1, sps)
                            pending.append((pt_A, pt_B, kc))
                            if len(pending) > 1:
                                issue_av(*pending.pop(0))
                        for args in pending:
                            issue_av(*args)
                        # merge halves + normalize: y.T = sum/Z. The copy/add
                        # also frees the PSUM banks without waiting on the
                        # broadcast DMA roundtrip.
                        for h, hb in ((hA, 0), (hB, 64)):
                            yh = npl.tile([65, QB], f32, tag="yh")
                            nc.scalar.activation(
                                yh[:], po[(h, 64)][:], AF.Identity)
                            yt = npl.tile([65, QB], f32, tag="yt")
                            nc.vector.tensor_add(yt[:], po[(h, 0)][:], yh[:])
                            rz = npl.tile([1, QB], f32, tag="rz")
                            nc.vector.reciprocal(rz[:], yt[64:65, :])
                            rz_dram = dram.tile([1, QB], f32, tag="rzd")
                            nc.gpsimd.dma_start(rz_dram[:], rz[:])
                            rzb = npl.tile([64, QB], f32, tag="rzb")
                            nc.gpsimd.dma_start(
                                rzb[:], rz_dram[:].to_broadcast([64, QB]))
                            nc.vector.tensor_mul(
                                y_sb[hb:hb + 64, p, q0:q0 + QB],
                                yt[0:64, :], rzb[:],
                            )

        # prefetch output-projection weights at program start (overlaps
        # earlier compute; avoids a load stall after attention)
        wo_sb = const.tile([128, NFC, D], mmdt)
        load_chunked(wo_sb, woT, NFC)

        def phase_out():
            with tc.tile_pool(name="owork", bufs=4) as owork, \
                 tc.tile_pool(name="opsum2", bufs=4, space="PSUM") as ops2:
                for dmc in range(NDC):
                    ob = owork.tile([128, N], mmdt, tag="ob")
                    for rn in range(N // 512):
                        ps_lo = ops2.tile([128, 512], f32, tag="olo")
                        ps_hi = ops2.tile([128, 512], f32, tag="ohi")
                        for fc in range(NFC):
                            for ps, r0 in ((ps_lo, 0), (ps_hi, 64)):
                                nc.tensor.matmul(
                                    ps[:],
                                    lhsT=wo_sb[r0:r0 + 64, fc,
                                               dmc * 128:(dmc + 1) * 128],
                                    rhs=y_sb[r0:r0 + 64, fc,
                                             rn * 512:(rn + 1) * 512],
                                    start=(fc == 0),
                                    stop=(fc == NFC - 1),
                                )
                        oh = owork.tile([128, 512], f32, tag="oh")
                        nc.scalar.activation(oh[:], ps_hi[:], AF.Identity)
                        nc.vector.scalar_tensor_tensor(
                            ob[:, rn * 512:(rn + 1) * 512], ps_lo[:],
                            bo_sb[:, dmc:dmc + 1], oh[:],
                            op0=ALU.add, op1=ALU.add)
                        # stream each 512-col stripe out as soon as it's done
                        dma_rr(
                            outT[dmc * 128:(dmc + 1) * 128,
                                 rn * 512:(rn + 1) * 512],
                            ob[:, rn * 512:(rn + 1) * 512])

        PHASES = int(os.environ.get("KPHASES", "9"))  # debug bisect knob

        def all_phases():
            if PHASES >= 1:
                phase_v()
            if PHASES >= 2:
                phase_q()
            if PHASES >= 3:
                phase_k()
            if PHASES >= 4:
                phase_attn()
            else:
                nc.vector.memset(y_sb[:], 0.0)
            if PHASES >= 5:
                phase_out()
            else:
                with tc.tile_pool(name="dummy", bufs=1) as dp:
                    zb = dp.tile([128, N], mmdt)
                    nc.vector.memset(zb[:], 0.0)
                    for dmc in range(NDC):
                        nc.sync.dma_start(
                            outT[dmc * 128:(dmc + 1) * 128, :], zb[:])

        if KLOOP > 1:
            with tc.For_i(0, KLOOP, 1):
                all_phases()
        else:
            all_phases()

    nc.compile()
    return nc


def _rope_tables(positions):
    """cos/sin tables [128, len(positions)] for the permuted transposed
    layout: partition p (within a 2-head feature chunk), j = p % 64:
    j<16: freq j (cos, -sin); 32<=j<48: freq j-32 (cos, +sin); else (1, 0)."""
    inv_freq = 1.0 / (THETA ** (np.arange(0, ROT, 2, dtype=np.float64) / ROT))  # [16]
    t = np.asarray(positions, dtype=np.float64)
    ang = t[None, :] * inv_freq[:, None]  # [16, nt]
    c, s = np.cos(ang), np.sin(ang)
    cos_tab = np.ones((128, len(positions)), dtype=np.float64)
    sin_tab = np.zeros((128, len(positions)), dtype=np.float64)
    for h2 in (0, 64):
        cos_tab[h2:h2 + 16] = c
        cos_tab[h2 + 32:h2 + 48] = c
        sin_tab[h2:h2 + 16] = -s
        sin_tab[h2 + 32:h2 + 48] = s
    return cos_tab.astype(np.float32), sin_tab.astype(np.float32)


def _head_perm(nfeat):
    """Feature permutation applied per head: within each head's 64 outputs
    -> [evens(16), pass 32:48, odds(16), pass 48:64]."""
    out = np.empty(nfeat, dtype=np.int64)
    for h in range(nfeat // DK):
        base = h * DK
        out[base:base + HALF] = base + np.arange(0, ROT, 2)
        out[base + HALF:base + ROT] = base + np.arange(ROT, ROT + HALF)
        out[base + ROT:base + ROT + HALF] = base + np.arange(1, ROT, 2)
        out[base + ROT + HALF:base + DK] = base + np.arange(ROT + HALF, DK)
    return out


def _prep_inputs(query, key, value, Wq, bq, Wk, bk, Wv, bv, Wo, bo,
                 mm_dtype_name="bfloat16"):
    import ml_dtypes

    np_mm = ml_dtypes.bfloat16 if mm_dtype_name == "bfloat16" else np.float32

    query = np.asarray(query, np.float32)
    key = np.asarray(key, np.float32)
    value = np.asarray(value, np.float32)
    Wq, bq = np.asarray(Wq, np.float32), np.asarray(bq, np.float32)
    Wk, bk = np.asarray(Wk, np.float32), np.asarray(bk, np.float32)
    Wv, bv = np.asarray(Wv, np.float32), np.asarray(bv, np.float32)
    Wo, bo = np.asarray(Wo, np.float32), np.asarray(bo, np.float32)

    perm = _head_perm(DH)
    cos_all, sin_all = _rope_tables(np.arange(N))
    # partner-permuted sin table (see sinPT comment in _build_program)
    pmap = np.arange(128)
    for h2 in (0, 64):
        pmap[h2:h2 + 16] = np.arange(h2 + 32, h2 + 48)
        pmap[h2 + 32:h2 + 48] = np.arange(h2, h2 + 16)
    sinP_all = sin_all[pmap]

    halves = []
    for hh in range(2):
        rows = slice(hh * DH, (hh + 1) * DH)  # head-feature rows of this half
        Wq_h, bq_h = Wq[rows][perm], bq[rows][perm]
        Wk_h, bk_h = Wk[rows][perm], bk[rows][perm]
        wqT = np.ascontiguousarray(Wq_h.T).astype(np_mm)
        wkT = np.ascontiguousarray(Wk_h.T).astype(np_mm)

        # W_v' : [D, HH*65] plus a separate bias/ones row wvb [1, HH*65]
        wvT = np.zeros((D, VW), np.float32)
        wvb = np.zeros((1, VW), np.float32)
        for h in range(HH):
            cols = slice(h * 65, h * 65 + 64)
            grows = slice(hh * DH + h * DK, hh * DH + (h + 1) * DK)
            wvT[:, cols] = Wv[grows, :].T
            wvb[0, cols] = bv[grows]
            wvb[0, h * 65 + 64] = 1.0

        # woT rows follow the y_sb layout: chunk fc holds heads (2fc, 2fc+1)
        wo_rows = np.empty((DH, D), np.float32)
        for fc in range(NFC):
            for sub in range(2):
                h = 2 * fc + sub
                grows = slice(hh * DH + h * DK, hh * DH + (h + 1) * DK)
                wo_rows[fc * 128 + sub * 64:fc * 128 + (sub + 1) * 64] = \
                    Wo[:, grows].T
        halves.append({
            "wqT": wqT, "wkT": wkT,
            "wvT": wvT.astype(np_mm), "wvb": wvb.astype(np_mm),
            "woT": np.ascontiguousarray(wo_rows).astype(np_mm),
            "bq_d": bq_h, "bk_d": bk_h,
            "bo_d": bo if hh == 0 else np.zeros_like(bo),
        })

    in_maps = []
    for core in range(NCORES):
        b, hh = core // 2, core % 2
        m = {
            "xqT": np.ascontiguousarray(query[b].T).astype(np_mm),
            "xkT": np.ascontiguousarray(key[b].T).astype(np_mm),
            "xvT": np.ascontiguousarray(value[b].T).astype(np_mm),
            "cosT": cos_all.astype(np_mm),
            "sinPT": sinP_all.astype(np_mm),
        }
        m.update(halves[hh])
        in_maps.append(m)
    return in_maps


def kernel(query, key, value, Wq, bq, Wk, bk, Wv, bv, Wo, bo):
    from concourse import bass_utils

    mm_dtype_name = "bfloat16"
    if mm_dtype_name not in _PROGRAM_CACHE:
        _PROGRAM_CACHE[mm_dtype_name] = _build_program(mm_dtype_name)
    nc = _PROGRAM_CACHE[mm_dtype_name]

    in_maps = _prep_inputs(query, key, value, Wq, bq, Wk, bk, Wv, bv, Wo, bo,
                           mm_dtype_name)

    res = bass_utils.run_bass_kernel_spmd(
        nc, in_maps, core_ids=list(range(NCORES))
    )

    out = np.empty((B, N, D), np.float32)
    for b in range(B):
        p0 = np.asarray(res.results[2 * b]["outT"], np.float32)
        p1 = np.asarray(res.results[2 * b + 1]["outT"], np.float32)
        out[b] = (p0 + p1).T
    return out
